# revision 16
# baseline (speedup 1.0000x reference)
"""Trainium2 Bass kernel for batched dense attention.

Problem: query/key/value [B=8, S=4096, D=128] fp32.
    logits = q @ k^T          (no scaling)
    attn   = softmax(logits, axis=-1)
    out    = attn @ v + v

Sharding: batch B=8 across the 8 NeuronCores (data parallel, no comms).

Per-core algorithm ("transposed attention", softmax over the partition axis),
v3 — ACT-saturating software pipeline:
    For each 512-query mega-block m, for each group g of 3 key-chunks
    (last group has 2):
      PSUM[k128, q1536] = K^T chunks . Q^T        (f32r matmuls, 3 banks)
      E^T group         = exp(PSUM) -> SBUF bf16  (one ACT instr, 1536 free)
      softmax partials: one bf16 DVE tensor_add of the E^T group into a
        running [128, 1536] accumulator (bf16 2x DVE mode);
        per-mega tail: 3 ones-matmuls partition-reduce it into PSUM.
      O^T[d, q512] += V^T . E^T      (bf16 PE matmuls, PSUM-accumulated)
    The PE stream is software-pipelined as QK(g+1) ; AV(g) so the Scalar
    (ACT) engine — the bottleneck at ~1.5us per 1536-wide exp — never
    waits behind AV's dependency on exp(g).
    Epilogue per mega (slotted into the next mega's PE gaps):
      out[q, d] = (O^T)^T * (1/sums)[q] + V[q, :]

Max-subtraction is skipped: logits ~ N(0, 128), |logit| < ~70 w.h.p., so
exp() stays inside fp32/bf16 range and the softmax ratio is unaffected.
bf16 E/V keep the end-to-end rel error ~1e-3 (gate is 2e-2).
"""

import numpy as np

B, S, D = 8, 4096, 128
N_CORES = 8
P = 128                 # partitions
QMEGA = 512             # queries per mega-block
N_MEGA = S // QMEGA     # 8
N_CHUNK = S // P        # 32 key chunks per core
# groups of 3 key-chunks per PSUM/exp step (last group of a mega has 2)
GRP_OF = [3] * 10 + [2]
N_GRP = len(GRP_OF)     # 11
GRP_OFF = [sum(GRP_OF[:i]) for i in range(N_GRP)]

_NC_CACHE = {}


def _patch_tile_drain(tile_mod):
    """Workaround for this walrus build rejecting >1-2 sem waits on the Tile
    tail Drain ("Too many sync wait commands"): spread the drain's waits
    across single-wait NOPs on the sync engine first."""
    if getattr(tile_mod.TileContext, "_drain_patched", False):
        return
    from concourse.vector_clock import ScopedClock
    from concourse import mybir

    def _drain_and_barrier(self, tick_clock, wait_clock):
        nc = self.nc
        probe = nc.sync.nop()
        wait_clock.add_sem_waits(
            probe.ins, ScopedClock({None: tick_clock.global_clock})
        )
        waits = (
            list(probe.ins.sync_info.on_wait or []) if probe.ins.sync_info else []
        )
        if probe.ins.sync_info is not None:
            probe.ins.sync_info.on_wait.clear()
        for w in waits:
            n = nc.sync.nop()
            n.ins.sync_info = mybir.SyncInfo(on_wait=[w], on_update=[])
        nc.sync.drain()

        nc.all_engine_barrier()
        assert self.sems is not None
        popped = nc._tile_sem_poison_stack.pop()
        assert popped is self._sem_poison
        nc.clear_and_free_semaphores(list(self.sems.allocated().values()))
        nc.all_engine_barrier()

    tile_mod.TileContext._drain_and_barrier = _drain_and_barrier
    tile_mod.TileContext._drain_patched = True


# This walrus build fits only ONE sync wait per emitted instruction
# (S3_LW matmuls and PSEUDO_DMA reject 2; Drain rejects 3) — cap at 1
# everywhere and carry excess waits on preceding same-engine NoOps.
_MAX_WAITS = 1
_MAX_WAITS_MATMUL = 1


def _split_excess_waits(nc):
    """Post-scheduling legalization: any instruction carrying more than
    the walrus per-instruction sync-wait limit gets same-engine NoOps
    inserted before it that carry the excess waits (the NX executes them
    in program order)."""
    from concourse import mybir

    uid = 0
    for fn in nc.m.functions:
        for bb in fn.blocks:
            new_insts = []
            for inst in bb.instructions:
                limit = (
                    _MAX_WAITS_MATMUL
                    if isinstance(inst, mybir.InstMatmult)
                    else _MAX_WAITS
                )
                si = inst.sync_info
                waits = list(si.on_wait) if (si and si.on_wait) else []
                if len(waits) > limit:
                    extra, keep = waits[:-limit], waits[-limit:]
                    for i in range(0, len(extra), _MAX_WAITS):
                        chunk = extra[i : i + _MAX_WAITS]
                        nop = mybir.InstNoOp(
                            name=f"I-waitsplit-{uid}", ins=[], outs=[]
                        )
                        uid += 1
                        nop.engine = inst.engine
                        nop.sync_info = mybir.SyncInfo(
                            on_wait=list(chunk), on_update=[]
                        )
                        new_insts.append(nop)
                    si.on_wait.clear()
                    si.on_wait.extend(keep)
                new_insts.append(inst)
            bb.instructions = new_insts


def _build_nc():
    if "nc" in _NC_CACHE:
        return _NC_CACHE["nc"]
    from contextlib import ExitStack

    import concourse.bass as bass
    import concourse.tile as tile
    from concourse import mybir
    from concourse.masks import make_identity

    _patch_tile_drain(tile)

    f32 = mybir.dt.float32
    f32r = mybir.dt.float32r
    bf16 = mybir.dt.bfloat16
    Exp = mybir.ActivationFunctionType.Exp

    nc = bass.Bass()
    q_d = nc.declare_dram_parameter("query", [S, D], f32, isOutput=False)
    k_d = nc.declare_dram_parameter("key", [S, D], f32, isOutput=False)
    v_d = nc.declare_dram_parameter("value", [S, D], f32, isOutput=False)
    o_d = nc.declare_dram_parameter("out", [S, D], f32, isOutput=True)

    with tile.TileContext(nc) as tc, ExitStack() as ctx:
        const = ctx.enter_context(tc.tile_pool(name="const", bufs=1))
        big = ctx.enter_context(tc.tile_pool(name="big", bufs=1))
        stage = ctx.enter_context(tc.tile_pool(name="stage", bufs=3))
        etp = ctx.enter_context(tc.tile_pool(name="et", bufs=6))
        accp = ctx.enter_context(tc.tile_pool(name="accp", bufs=2))
        outp = ctx.enter_context(tc.tile_pool(name="outp", bufs=6))
        smallp = ctx.enter_context(tc.tile_pool(name="small", bufs=4))
        # PSUM: gp 2x6KB + acc 1x2KB + shared transpose/sums 1x2KB = 16KB
        grp_ps = ctx.enter_context(tc.tile_pool(name="grp_ps", bufs=2, space="PSUM"))
        acc_ps = ctx.enter_context(tc.tile_pool(name="acc_ps", bufs=1, space="PSUM"))
        o_ps = ctx.enter_context(tc.tile_pool(name="o_ps", bufs=1, space="PSUM"))

        # The very first Q/K slab DMAs go out via gpsimd's software DGE
        # BEFORE any other gpsimd work (identity/casts): the gpsimd
        # preamble retires ~1.3us before the sync engine's, so the first
        # slabs land earlier and the whole pipeline starts sooner.
        st_q0 = stage.tile([P, 4, P], f32, tag="stage")
        nc.gpsimd.dma_start(
            out=st_q0, in_=q_d[0:512, :].rearrange("(n p) d -> p n d", p=P)
        )
        st_k0 = stage.tile([P, 4, P], f32, tag="stage")
        nc.gpsimd.dma_start(
            out=st_k0, in_=k_d[0:512, :].rearrange("(n p) d -> p n d", p=P)
        )

        ident = const.tile([P, P], f32)
        make_identity(nc, ident)
        ones_f32 = const.tile([P, 1], f32)
        nc.vector.memset(ones_f32, 1.0)
        ones_bf = const.tile([P, 1], bf16)
        nc.vector.tensor_copy(ones_bf, ones_f32)

        # V resident in natural layout: vt[p, n, d] = V[n*128 + p, d]
        # (used for the +V epilogue), and vtr bf16 for the AV matmuls
        # (cast on the otherwise-idle Pool engine).
        vt = big.tile([P, N_CHUNK, P], f32)
        vtr = big.tile([P, N_CHUNK, P], bf16)
        v_re = v_d.rearrange("(n p) d -> p n d", p=P)

        def load_v_piece(i):
            sl = slice(i * 4, (i + 1) * 4)
            nc.sync.dma_start(out=vt[:, sl, :], in_=v_re[:, sl, :])
            nc.gpsimd.tensor_copy(vtr[:, sl, :], vt[:, sl, :])

        # K^T / Q^T [d, s] via PE transposes of natural [s, d] tiles.
        qt = big.tile([P, S], f32r)
        kt = big.tile([P, S], f32r)

        def transpose_512(src_ap, dst, r, st=None):
            """dst[:, r*512:(r+1)*512] = src_ap[r*512:(r+1)*512, :].T"""
            if st is None:
                st = stage.tile([P, 4, P], f32, tag="stage")
                nc.sync.dma_start(
                    out=st,
                    in_=src_ap[r * 512 : (r + 1) * 512, :].rearrange(
                        "(n p) d -> p n d", p=P
                    ),
                )
            ops = o_ps.tile([P, 512], f32, tag="ops")
            for t in range(4):
                nc.tensor.transpose(ops[:, t * P : (t + 1) * P], st[:, t, :], ident)
            nc.vector.tensor_copy(dst[:, r * 512 : (r + 1) * 512], ops)

        # Q^T for mega 0 and K round 0 first so mega 0's matmuls can
        # start while V and the later K rounds are still arriving.
        transpose_512(q_d, qt, 0, st=st_q0)
        transpose_512(k_d, kt, 0, st=st_k0)
        for r in range(1, S // 512):
            load_v_piece(r - 1)
            transpose_512(k_d, kt, r)
        load_v_piece(7)

        pending_ot = None
        pending_fold = None
        pending_epi_a = None
        pending_epi_b = None
        prev_av = None

        for m in range(N_MEGA):
            qs = slice(m * QMEGA, (m + 1) * QMEGA)
            acc = None
            acc_d = accp.tile([P, 1536], bf16, tag="acc_d")
            et0 = None

            for g in range(N_GRP):
                width = GRP_OF[g] * 512
                # --- PE: QK matmuls for group g (before AV(g-1)) ---
                gp = grp_ps.tile([P, 1536], f32, tag="grp")
                for j in range(GRP_OF[g]):
                    kc = GRP_OFF[g] + j
                    nc.tensor.matmul(
                        gp[:, j * 512 : (j + 1) * 512],
                        lhsT=kt[:, kc * P : (kc + 1) * P],
                        rhs=qt[:, qs],
                        start=True,
                        stop=True,
                    )
                # --- ACT: exp -> bf16 SBUF ---
                et = etp.tile([P, 1536], bf16, tag="et")
                nc.scalar.activation(et[:, :width], gp[:, :width], Exp)
                # --- softmax partial sums: bf16 DVE accumulate ---
                if g == 0:
                    et0 = et
                elif g == 1:
                    nc.vector.tensor_add(acc_d, et0, et)
                elif g < N_GRP - 1:
                    nc.vector.tensor_add(acc_d, acc_d, et)
                else:
                    nc.vector.tensor_add(
                        acc_d[:, :width], acc_d[:, :width], et[:, :width]
                    )
                # --- PE: AV matmuls for the previous group (software
                # pipeline, carried across the mega boundary so the next
                # mega's QK never waits behind the last AV group) ---
                if prev_av is not None:
                    prev_av()
                    prev_av = None
                if acc is None:
                    # Emitted only after the previous mega's last AV write:
                    # first drain the previous acc (O^T copy), then create
                    # this mega's acc tile, keeping acc-pool tile creation
                    # in consumption order (the pool has a single buffer).
                    if pending_ot is not None:
                        pending_ot()
                        pending_ot = None
                    acc = acc_ps.tile([P, QMEGA], f32, tag="acc")

                def make_av(g, et, acc):
                    def av():
                        for j in range(GRP_OF[g]):
                            kc = GRP_OFF[g] + j
                            nc.tensor.matmul(
                                acc,
                                lhsT=vtr[:, kc, :],
                                rhs=et[:, j * 512 : (j + 1) * 512],
                                start=(kc == 0),
                                stop=(kc == N_CHUNK - 1),
                                skip_group_check=True,
                            )

                    return av

                prev_av = make_av(g, et, acc)

                # --- per-mega specials slotted into PE gaps ---
                if g == 2 and pending_fold is not None:
                    pending_fold()
                    pending_fold = None
                if g == 4 and pending_epi_a is not None:
                    pending_epi_a()
                    pending_epi_a = None
                if g == 6 and pending_epi_b is not None:
                    pending_epi_b()
                    pending_epi_b = None
                if g == 8 and m + 1 < N_MEGA:
                    transpose_512(q_d, qt, m + 1)

            def make_stages(m, acc_d, acc):
                state = {}

                def ot_copy():
                    ot_sb = outp.tile([P, QMEGA], f32, tag="ot")
                    nc.vector.tensor_copy(ot_sb, acc)
                    state["ot"] = ot_sb

                def fold():
                    # partition-reduce the bf16 partials: 3 ones-matmuls
                    sums = o_ps.tile([1, QMEGA], f32, tag="ops")
                    for j in range(3):
                        nc.tensor.matmul(
                            sums,
                            lhsT=ones_bf,
                            rhs=acc_d[:, j * 512 : (j + 1) * 512],
                            start=(j == 0),
                            stop=(j == 2),
                            skip_group_check=True,
                        )
                    sums_sb = smallp.tile([1, QMEGA], f32, tag="sums_sb")
                    nc.vector.tensor_copy(sums_sb, sums)
                    state["sums"] = sums_sb

                def epi_a():
                    sums_sb = state["sums"]
                    # 1/sums: [1, 512] -> [128, 4] per-partition scalars
                    rt = o_ps.tile([P, 4], f32, tag="ops")
                    for t in range(4):
                        nc.tensor.transpose(
                            rt[:, t : t + 1],
                            sums_sb[0:1, t * P : (t + 1) * P],
                            ident[0:1, 0:1],
                        )
                    recip = smallp.tile([P, 4], f32, tag="recip")
                    nc.vector.reciprocal(recip, rt)
                    state["recip"] = recip

                def epi_b():
                    ot_sb = state["ot"]
                    recip = state["recip"]
                    # O^T -> O, normalize, +V, one batched store per mega
                    ops2 = o_ps.tile([P, 512], f32, tag="ops")
                    for t in range(4):
                        nc.tensor.transpose(
                            ops2[:, t * P : (t + 1) * P],
                            ot_sb[:, t * P : (t + 1) * P],
                            ident,
                        )
                    o_sb = outp.tile([P, 4, P], f32, tag="osb")
                    for t in range(4):
                        nc.vector.scalar_tensor_tensor(
                            o_sb[:, t, :],
                            ops2[:, t * P : (t + 1) * P],
                            recip[:, t : t + 1],
                            vt[:, m * 4 + t, :],
                            mybir.AluOpType.mult,
                            mybir.AluOpType.add,
                        )
                    nc.sync.dma_start(
                        out=o_d[m * QMEGA : (m + 1) * QMEGA, :].rearrange(
                            "(n p) d -> p n d", p=P
                        ),
                        in_=o_sb,
                    )

                return ot_copy, fold, epi_a, epi_b

            pending_ot, pending_fold, pending_epi_a, pending_epi_b = make_stages(
                m, acc_d, acc
            )
        prev_av()
        pending_ot()
        pending_fold()
        pending_epi_a()
        pending_epi_b()

    _split_excess_waits(nc)
    _NC_CACHE["nc"] = nc
    return nc


def kernel_run(inputs, trace=False):
    from concourse.bass_utils import run_bass_kernel_spmd

    query = np.ascontiguousarray(inputs["query"], dtype=np.float32)
    key = np.ascontiguousarray(inputs["key"], dtype=np.float32)
    value = np.ascontiguousarray(inputs["value"], dtype=np.float32)
    assert query.shape == (B, S, D), query.shape

    nc = _build_nc()
    in_maps = [
        {
            "query": np.ascontiguousarray(query[c]),
            "key": np.ascontiguousarray(key[c]),
            "value": np.ascontiguousarray(value[c]),
        }
        for c in range(N_CORES)
    ]
    res = run_bass_kernel_spmd(nc, in_maps, list(range(N_CORES)), trace=trace)
    out = np.stack([res.results[c]["out"] for c in range(N_CORES)], axis=0)
    return out.astype(np.float32), res


def kernel(**inputs) -> np.ndarray:
    out, _ = kernel_run(inputs, trace=False)
    return out


# revision 18
# speedup vs baseline: 1.2120x; 1.2120x over previous
"""Trainium2 Bass kernel for batched dense attention.

Problem: query/key/value [B=8, S=4096, D=128] fp32.
    logits = q @ k^T          (no scaling)
    attn   = softmax(logits, axis=-1)
    out    = attn @ v + v

Sharding: batch B=8 across the 8 NeuronCores (data parallel, no comms).

Per-core algorithm ("transposed attention", softmax over the partition axis),
v3 — ACT-saturating software pipeline:
    For each 512-query mega-block m, for each group g of 3 key-chunks
    (last group has 2):
      PSUM[k128, q1536] = K^T chunks . Q^T        (f32r matmuls, 3 banks)
      E^T group         = exp(PSUM) -> SBUF bf16  (one ACT instr, 1536 free)
      softmax partials: one bf16 DVE tensor_add of the E^T group into a
        running [128, 1536] accumulator (bf16 2x DVE mode);
        per-mega tail: 3 ones-matmuls partition-reduce it into PSUM.
      O^T[d, q512] += V^T . E^T      (bf16 PE matmuls, PSUM-accumulated)
    The PE stream is software-pipelined as QK(g+1) ; AV(g) so the Scalar
    (ACT) engine — the bottleneck at ~1.5us per 1536-wide exp — never
    waits behind AV's dependency on exp(g).
    Epilogue per mega (slotted into the next mega's PE gaps):
      out[q, d] = (O^T)^T * (1/sums)[q] + V[q, :]

Max-subtraction is skipped: logits ~ N(0, 128), |logit| < ~70 w.h.p., so
exp() stays inside fp32/bf16 range and the softmax ratio is unaffected.
bf16 E/V keep the end-to-end rel error ~1e-3 (gate is 2e-2).
"""

import numpy as np

B, S, D = 8, 4096, 128
N_CORES = 8
P = 128                 # partitions
QMEGA = 512             # queries per mega-block
N_MEGA = S // QMEGA     # 8
N_CHUNK = S // P        # 32 key chunks per core
# groups of 3 key-chunks per PSUM/exp step (last group of a mega has 2)
GRP_OF = [3] * 10 + [2]
N_GRP = len(GRP_OF)     # 11
GRP_OFF = [sum(GRP_OF[:i]) for i in range(N_GRP)]

_NC_CACHE = {}


def _patch_tile_drain(tile_mod):
    """Workaround for this walrus build rejecting >1-2 sem waits on the Tile
    tail Drain ("Too many sync wait commands"): spread the drain's waits
    across single-wait NOPs on the sync engine first."""
    if getattr(tile_mod.TileContext, "_drain_patched", False):
        return
    from concourse.vector_clock import ScopedClock
    from concourse import mybir

    def _drain_and_barrier(self, tick_clock, wait_clock):
        nc = self.nc
        probe = nc.sync.nop()
        wait_clock.add_sem_waits(
            probe.ins, ScopedClock({None: tick_clock.global_clock})
        )
        waits = (
            list(probe.ins.sync_info.on_wait or []) if probe.ins.sync_info else []
        )
        if probe.ins.sync_info is not None:
            probe.ins.sync_info.on_wait.clear()
        for w in waits:
            n = nc.sync.nop()
            n.ins.sync_info = mybir.SyncInfo(on_wait=[w], on_update=[])
        nc.sync.drain()

        nc.all_engine_barrier()
        assert self.sems is not None
        popped = nc._tile_sem_poison_stack.pop()
        assert popped is self._sem_poison
        nc.clear_and_free_semaphores(list(self.sems.allocated().values()))
        nc.all_engine_barrier()

    tile_mod.TileContext._drain_and_barrier = _drain_and_barrier
    tile_mod.TileContext._drain_patched = True


# This walrus build fits only ONE sync wait per emitted instruction
# (S3_LW matmuls and PSEUDO_DMA reject 2; Drain rejects 3) — cap at 1
# everywhere and carry excess waits on preceding same-engine NoOps.
_MAX_WAITS = 1
_MAX_WAITS_MATMUL = 1


def _split_excess_waits(nc):
    """Post-scheduling legalization: any instruction carrying more than
    the walrus per-instruction sync-wait limit gets same-engine NoOps
    inserted before it that carry the excess waits (the NX executes them
    in program order)."""
    from concourse import mybir

    uid = 0
    for fn in nc.m.functions:
        for bb in fn.blocks:
            new_insts = []
            for inst in bb.instructions:
                limit = (
                    _MAX_WAITS_MATMUL
                    if isinstance(inst, mybir.InstMatmult)
                    else _MAX_WAITS
                )
                si = inst.sync_info
                waits = list(si.on_wait) if (si and si.on_wait) else []
                if len(waits) > limit:
                    extra, keep = waits[:-limit], waits[-limit:]
                    for i in range(0, len(extra), _MAX_WAITS):
                        chunk = extra[i : i + _MAX_WAITS]
                        nop = mybir.InstNoOp(
                            name=f"I-waitsplit-{uid}", ins=[], outs=[]
                        )
                        uid += 1
                        nop.engine = inst.engine
                        nop.sync_info = mybir.SyncInfo(
                            on_wait=list(chunk), on_update=[]
                        )
                        new_insts.append(nop)
                    si.on_wait.clear()
                    si.on_wait.extend(keep)
                new_insts.append(inst)
            bb.instructions = new_insts


def _build_nc():
    if "nc" in _NC_CACHE:
        return _NC_CACHE["nc"]
    from contextlib import ExitStack

    import concourse.bass as bass
    import concourse.tile as tile
    from concourse import mybir
    from concourse.masks import make_identity

    _patch_tile_drain(tile)

    f32 = mybir.dt.float32
    f32r = mybir.dt.float32r
    bf16 = mybir.dt.bfloat16
    Exp = mybir.ActivationFunctionType.Exp

    nc = bass.Bass()
    q_d = nc.declare_dram_parameter("query", [S, D], f32, isOutput=False)
    k_d = nc.declare_dram_parameter("key", [S, D], f32, isOutput=False)
    v_d = nc.declare_dram_parameter("value", [S, D], f32, isOutput=False)
    o_d = nc.declare_dram_parameter("out", [S, D], f32, isOutput=True)

    with tile.TileContext(nc) as tc, ExitStack() as ctx:
        const = ctx.enter_context(tc.tile_pool(name="const", bufs=1))
        big = ctx.enter_context(tc.tile_pool(name="big", bufs=1))
        stage = ctx.enter_context(tc.tile_pool(name="stage", bufs=3))
        etp = ctx.enter_context(tc.tile_pool(name="et", bufs=6))
        accp = ctx.enter_context(tc.tile_pool(name="accp", bufs=2))
        outp = ctx.enter_context(tc.tile_pool(name="outp", bufs=6))
        smallp = ctx.enter_context(tc.tile_pool(name="small", bufs=4))
        # PSUM: gp 2x6KB + acc 1x2KB + shared transpose/sums 1x2KB = 16KB
        grp_ps = ctx.enter_context(tc.tile_pool(name="grp_ps", bufs=2, space="PSUM"))
        acc_ps = ctx.enter_context(tc.tile_pool(name="acc_ps", bufs=1, space="PSUM"))
        o_ps = ctx.enter_context(tc.tile_pool(name="o_ps", bufs=1, space="PSUM"))

        ident = const.tile([P, P], f32)
        make_identity(nc, ident)
        ones_f32 = const.tile([P, 1], f32)
        nc.vector.memset(ones_f32, 1.0)
        ones_bf = const.tile([P, 1], bf16)
        nc.vector.tensor_copy(ones_bf, ones_f32)

        # V resident in natural layout: vt[p, n, d] = V[n*128 + p, d]
        # (used for the +V epilogue), and vtr bf16 for the AV matmuls
        # (cast on the otherwise-idle Pool engine).
        vt = big.tile([P, N_CHUNK, P], f32)
        vtr = big.tile([P, N_CHUNK, P], bf16)
        v_re = v_d.rearrange("(n p) d -> p n d", p=P)

        def load_v_piece(i):
            sl = slice(i * 4, (i + 1) * 4)
            nc.sync.dma_start(out=vt[:, sl, :], in_=v_re[:, sl, :])
            nc.gpsimd.tensor_copy(vtr[:, sl, :], vt[:, sl, :])

        # K^T / Q^T [d, s] via PE transposes of natural [s, d] tiles.
        qt = big.tile([P, S], f32r)
        kt = big.tile([P, S], f32r)

        def transpose_512(src_ap, dst, r):
            """dst[:, r*512:(r+1)*512] = src_ap[r*512:(r+1)*512, :].T"""
            st = stage.tile([P, 4, P], f32, tag="stage")
            nc.sync.dma_start(
                out=st,
                in_=src_ap[r * 512 : (r + 1) * 512, :].rearrange(
                    "(n p) d -> p n d", p=P
                ),
            )
            ops = o_ps.tile([P, 512], f32, tag="ops")
            for t in range(4):
                nc.tensor.transpose(ops[:, t * P : (t + 1) * P], st[:, t, :], ident)
            nc.vector.tensor_copy(dst[:, r * 512 : (r + 1) * 512], ops)

        # Q^T for mega 0 and K round 0 first so mega 0's matmuls can
        # start while V and the later K rounds are still arriving.
        transpose_512(q_d, qt, 0)
        transpose_512(k_d, kt, 0)
        for r in range(1, S // 512):
            load_v_piece(r - 1)
            transpose_512(k_d, kt, r)
        load_v_piece(7)

        pending_ot = None
        pending_fold = None
        pending_epi_a = None
        pending_epi_b = None
        prev_av = None

        for m in range(N_MEGA):
            qs = slice(m * QMEGA, (m + 1) * QMEGA)
            acc = None
            acc_d = accp.tile([P, 1536], bf16, tag="acc_d")
            et0 = None

            for g in range(N_GRP):
                width = GRP_OF[g] * 512
                # --- PE: QK matmuls for group g (before AV(g-1)) ---
                gp = grp_ps.tile([P, 1536], f32, tag="grp")
                for j in range(GRP_OF[g]):
                    kc = GRP_OFF[g] + j
                    nc.tensor.matmul(
                        gp[:, j * 512 : (j + 1) * 512],
                        lhsT=kt[:, kc * P : (kc + 1) * P],
                        rhs=qt[:, qs],
                        start=True,
                        stop=True,
                    )
                # --- ACT: exp -> bf16 SBUF ---
                et = etp.tile([P, 1536], bf16, tag="et")
                nc.scalar.activation(et[:, :width], gp[:, :width], Exp)
                # --- softmax partial sums: bf16 DVE accumulate ---
                if g == 0:
                    et0 = et
                elif g == 1:
                    nc.vector.tensor_add(acc_d, et0, et)
                elif g < N_GRP - 1:
                    nc.vector.tensor_add(acc_d, acc_d, et)
                else:
                    nc.vector.tensor_add(
                        acc_d[:, :width], acc_d[:, :width], et[:, :width]
                    )
                # --- PE: AV matmuls for the previous group (software
                # pipeline, carried across the mega boundary so the next
                # mega's QK never waits behind the last AV group) ---
                if prev_av is not None:
                    prev_av()
                    prev_av = None
                if acc is None:
                    # Emitted only after the previous mega's last AV write:
                    # first drain the previous acc (O^T copy), then create
                    # this mega's acc tile, keeping acc-pool tile creation
                    # in consumption order (the pool has a single buffer).
                    if pending_ot is not None:
                        pending_ot()
                        pending_ot = None
                    acc = acc_ps.tile([P, QMEGA], f32, tag="acc")

                def make_av(g, et, acc):
                    def av():
                        for j in range(GRP_OF[g]):
                            kc = GRP_OFF[g] + j
                            nc.tensor.matmul(
                                acc,
                                lhsT=vtr[:, kc, :],
                                rhs=et[:, j * 512 : (j + 1) * 512],
                                start=(kc == 0),
                                stop=(kc == N_CHUNK - 1),
                                skip_group_check=True,
                            )

                    return av

                prev_av = make_av(g, et, acc)

                # --- per-mega specials slotted into PE gaps ---
                if g == 2 and pending_fold is not None:
                    pending_fold()
                    pending_fold = None
                if g == 4 and pending_epi_a is not None:
                    pending_epi_a()
                    pending_epi_a = None
                if g == 6 and pending_epi_b is not None:
                    pending_epi_b()
                    pending_epi_b = None
                if g == 8 and m + 1 < N_MEGA:
                    transpose_512(q_d, qt, m + 1)

            def make_stages(m, acc_d, acc):
                state = {}

                def ot_copy():
                    ot_sb = outp.tile([P, QMEGA], f32, tag="ot")
                    nc.vector.tensor_copy(ot_sb, acc)
                    state["ot"] = ot_sb

                def fold():
                    # partition-reduce the bf16 partials: 3 ones-matmuls
                    sums = o_ps.tile([1, QMEGA], f32, tag="ops")
                    for j in range(3):
                        nc.tensor.matmul(
                            sums,
                            lhsT=ones_bf,
                            rhs=acc_d[:, j * 512 : (j + 1) * 512],
                            start=(j == 0),
                            stop=(j == 2),
                            skip_group_check=True,
                        )
                    sums_sb = smallp.tile([1, QMEGA], f32, tag="sums_sb")
                    nc.vector.tensor_copy(sums_sb, sums)
                    state["sums"] = sums_sb

                def epi_a():
                    sums_sb = state["sums"]
                    # 1/sums: [1, 512] -> [128, 4] per-partition scalars
                    rt = o_ps.tile([P, 4], f32, tag="ops")
                    for t in range(4):
                        nc.tensor.transpose(
                            rt[:, t : t + 1],
                            sums_sb[0:1, t * P : (t + 1) * P],
                            ident[0:1, 0:1],
                        )
                    recip = smallp.tile([P, 4], f32, tag="recip")
                    nc.vector.reciprocal(recip, rt)
                    state["recip"] = recip

                def epi_b():
                    ot_sb = state["ot"]
                    recip = state["recip"]
                    # O^T -> O, normalize, +V, one batched store per mega
                    ops2 = o_ps.tile([P, 512], f32, tag="ops")
                    for t in range(4):
                        nc.tensor.transpose(
                            ops2[:, t * P : (t + 1) * P],
                            ot_sb[:, t * P : (t + 1) * P],
                            ident,
                        )
                    o_sb = outp.tile([P, 4, P], f32, tag="osb")
                    for t in range(4):
                        nc.vector.scalar_tensor_tensor(
                            o_sb[:, t, :],
                            ops2[:, t * P : (t + 1) * P],
                            recip[:, t : t + 1],
                            vt[:, m * 4 + t, :],
                            mybir.AluOpType.mult,
                            mybir.AluOpType.add,
                        )
                    nc.sync.dma_start(
                        out=o_d[m * QMEGA : (m + 1) * QMEGA, :].rearrange(
                            "(n p) d -> p n d", p=P
                        ),
                        in_=o_sb,
                    )

                return ot_copy, fold, epi_a, epi_b

            pending_ot, pending_fold, pending_epi_a, pending_epi_b = make_stages(
                m, acc_d, acc
            )
        prev_av()
        pending_ot()
        pending_fold()
        pending_epi_a()
        pending_epi_b()

    _split_excess_waits(nc)
    _NC_CACHE["nc"] = nc
    return nc


def kernel_run(inputs, trace=False):
    from concourse.bass_utils import run_bass_kernel_spmd

    query = np.ascontiguousarray(inputs["query"], dtype=np.float32)
    key = np.ascontiguousarray(inputs["key"], dtype=np.float32)
    value = np.ascontiguousarray(inputs["value"], dtype=np.float32)
    assert query.shape == (B, S, D), query.shape

    nc = _build_nc()
    in_maps = [
        {
            "query": np.ascontiguousarray(query[c]),
            "key": np.ascontiguousarray(key[c]),
            "value": np.ascontiguousarray(value[c]),
        }
        for c in range(N_CORES)
    ]
    res = run_bass_kernel_spmd(nc, in_maps, list(range(N_CORES)), trace=trace)
    out = np.stack([res.results[c]["out"] for c in range(N_CORES)], axis=0)
    return out.astype(np.float32), res


def kernel(**inputs) -> np.ndarray:
    out, _ = kernel_run(inputs, trace=False)
    return out


# revision 28
# speedup vs baseline: 1.2220x; 1.0083x over previous
"""Trainium2 Bass kernel for batched dense attention.

Problem: query/key/value [B=8, S=4096, D=128] fp32.
    logits = q @ k^T          (no scaling)
    attn   = softmax(logits, axis=-1)
    out    = attn @ v + v

Sharding: batch B=8 across the 8 NeuronCores (data parallel, no comms).

Per-core algorithm ("transposed attention", softmax over the partition axis),
v3 — ACT-saturating software pipeline:
    For each 512-query mega-block m, for each group g of 3 key-chunks
    (last group has 2):
      PSUM[k128, q1536] = K^T chunks . Q^T        (f32r matmuls, 3 banks)
      E^T group         = exp(PSUM) -> SBUF bf16  (one ACT instr, 1536 free)
      softmax partials: one bf16 DVE tensor_add of the E^T group into a
        running [128, 1536] accumulator (bf16 2x DVE mode);
        per-mega tail: 3 ones-matmuls partition-reduce it into PSUM.
      O^T[d, q512] += V^T . E^T      (bf16 PE matmuls, PSUM-accumulated)
    The PE stream is software-pipelined as QK(g+1) ; AV(g) so the Scalar
    (ACT) engine — the bottleneck at ~1.5us per 1536-wide exp — never
    waits behind AV's dependency on exp(g).
    Epilogue per mega (slotted into the next mega's PE gaps):
      out[q, d] = (O^T)^T * (1/sums)[q] + V[q, :]

Max-subtraction is skipped: logits ~ N(0, 128), |logit| < ~70 w.h.p., so
exp() stays inside fp32/bf16 range and the softmax ratio is unaffected.
bf16 E/V keep the end-to-end rel error ~1e-3 (gate is 2e-2).
"""

import numpy as np

B, S, D = 8, 4096, 128
N_CORES = 8
P = 128                 # partitions
QMEGA = 512             # queries per mega-block
N_MEGA = S // QMEGA     # 8
N_CHUNK = S // P        # 32 key chunks per core
# groups of key-chunks per PSUM/exp step. The short group leads each
# mega: the first QK then only needs the first 256 K rows (fast start),
# and every mega-boundary exp window is a full-width 1536 one, wide
# enough to cover the AV+QK tensor work scheduled under it.
GRP_OF = [2] + [3] * 10
N_GRP = len(GRP_OF)     # 11
GRP_OFF = [sum(GRP_OF[:i]) for i in range(N_GRP)]

_NC_CACHE = {}


def _patch_tile_drain(tile_mod):
    """Workaround for this walrus build rejecting >1-2 sem waits on the Tile
    tail Drain ("Too many sync wait commands"): spread the drain's waits
    across single-wait NOPs on the sync engine first."""
    if getattr(tile_mod.TileContext, "_drain_patched", False):
        return
    from concourse.vector_clock import ScopedClock
    from concourse import mybir

    def _drain_and_barrier(self, tick_clock, wait_clock):
        nc = self.nc
        probe = nc.sync.nop()
        wait_clock.add_sem_waits(
            probe.ins, ScopedClock({None: tick_clock.global_clock})
        )
        waits = (
            list(probe.ins.sync_info.on_wait or []) if probe.ins.sync_info else []
        )
        if probe.ins.sync_info is not None:
            probe.ins.sync_info.on_wait.clear()
        for w in waits:
            n = nc.sync.nop()
            n.ins.sync_info = mybir.SyncInfo(on_wait=[w], on_update=[])
        nc.sync.drain()

        nc.all_engine_barrier()
        assert self.sems is not None
        popped = nc._tile_sem_poison_stack.pop()
        assert popped is self._sem_poison
        nc.clear_and_free_semaphores(list(self.sems.allocated().values()))
        nc.all_engine_barrier()

    tile_mod.TileContext._drain_and_barrier = _drain_and_barrier
    tile_mod.TileContext._drain_patched = True


# This walrus build fits only ONE sync wait per emitted instruction
# (S3_LW matmuls and PSEUDO_DMA reject 2; Drain rejects 3) — cap at 1
# everywhere and carry excess waits on preceding same-engine NoOps.
_MAX_WAITS = 1
_MAX_WAITS_MATMUL = 1


def _split_excess_waits(nc):
    """Post-scheduling legalization: any instruction carrying more than
    the walrus per-instruction sync-wait limit gets same-engine NoOps
    inserted before it that carry the excess waits (the NX executes them
    in program order)."""
    from concourse import mybir

    uid = 0
    for fn in nc.m.functions:
        for bb in fn.blocks:
            new_insts = []
            for inst in bb.instructions:
                limit = (
                    _MAX_WAITS_MATMUL
                    if isinstance(inst, mybir.InstMatmult)
                    else _MAX_WAITS
                )
                si = inst.sync_info
                waits = list(si.on_wait) if (si and si.on_wait) else []
                if len(waits) > limit:
                    extra, keep = waits[:-limit], waits[-limit:]
                    for i in range(0, len(extra), _MAX_WAITS):
                        chunk = extra[i : i + _MAX_WAITS]
                        nop = mybir.InstNoOp(
                            name=f"I-waitsplit-{uid}", ins=[], outs=[]
                        )
                        uid += 1
                        nop.engine = inst.engine
                        nop.sync_info = mybir.SyncInfo(
                            on_wait=list(chunk), on_update=[]
                        )
                        new_insts.append(nop)
                    si.on_wait.clear()
                    si.on_wait.extend(keep)
                new_insts.append(inst)
            bb.instructions = new_insts


def _build_nc():
    if "nc" in _NC_CACHE:
        return _NC_CACHE["nc"]
    from contextlib import ExitStack

    import concourse.bass as bass
    import concourse.tile as tile
    from concourse import mybir
    from concourse.masks import make_identity

    _patch_tile_drain(tile)

    f32 = mybir.dt.float32
    f32r = mybir.dt.float32r
    bf16 = mybir.dt.bfloat16
    Exp = mybir.ActivationFunctionType.Exp

    nc = bass.Bass()
    q_d = nc.declare_dram_parameter("query", [S, D], f32, isOutput=False)
    k_d = nc.declare_dram_parameter("key", [S, D], f32, isOutput=False)
    v_d = nc.declare_dram_parameter("value", [S, D], f32, isOutput=False)
    o_d = nc.declare_dram_parameter("out", [S, D], f32, isOutput=True)

    with tile.TileContext(nc) as tc, ExitStack() as ctx:
        const = ctx.enter_context(tc.tile_pool(name="const", bufs=1))
        big = ctx.enter_context(tc.tile_pool(name="big", bufs=1))
        stage = ctx.enter_context(tc.tile_pool(name="stage", bufs=3))
        etp = ctx.enter_context(tc.tile_pool(name="et", bufs=6))
        accp = ctx.enter_context(tc.tile_pool(name="accp", bufs=2))
        outp = ctx.enter_context(tc.tile_pool(name="outp", bufs=6))
        smallp = ctx.enter_context(tc.tile_pool(name="small", bufs=4))
        # PSUM: gp 2x6KB + acc 1x2KB + shared transpose/sums 1x2KB = 16KB
        grp_ps = ctx.enter_context(tc.tile_pool(name="grp_ps", bufs=2, space="PSUM"))
        acc_ps = ctx.enter_context(tc.tile_pool(name="acc_ps", bufs=1, space="PSUM"))
        o_ps = ctx.enter_context(tc.tile_pool(name="o_ps", bufs=1, space="PSUM"))

        ident = const.tile([P, P], f32)
        make_identity(nc, ident)
        ones_f32 = const.tile([P, 1], f32)
        nc.vector.memset(ones_f32, 1.0)
        ones_bf = const.tile([P, 1], bf16)
        nc.vector.tensor_copy(ones_bf, ones_f32)

        # V resident in natural layout: vt[p, n, d] = V[n*128 + p, d]
        # (used for the +V epilogue), and vtr bf16 for the AV matmuls
        # (cast on the otherwise-idle Pool engine).
        vt = big.tile([P, N_CHUNK, P], f32)
        vtr = big.tile([P, N_CHUNK, P], bf16)
        v_re = v_d.rearrange("(n p) d -> p n d", p=P)

        def load_v_piece(i):
            sl = slice(i * 4, (i + 1) * 4)
            nc.sync.dma_start(out=vt[:, sl, :], in_=v_re[:, sl, :])
            nc.gpsimd.tensor_copy(vtr[:, sl, :], vt[:, sl, :])

        # K^T / Q^T [d, s] via PE transposes of natural [s, d] tiles.
        qt = big.tile([P, S], f32r)
        kt = big.tile([P, S], f32r)

        def transpose_rows(src_ap, dst, row0, nrows):
            """dst[:, row0:row0+nrows] = src_ap[row0:row0+nrows, :].T"""
            n = nrows // P
            st = stage.tile([P, n, P], f32, tag="stage", name=f"st{row0}")
            nc.sync.dma_start(
                out=st,
                in_=src_ap[row0 : row0 + nrows, :].rearrange(
                    "(n p) d -> p n d", p=P
                ),
            )
            ops = o_ps.tile([P, 512], f32, tag="ops", name=f"ops{row0}")
            for t in range(n):
                nc.tensor.transpose(ops[:, t * P : (t + 1) * P], st[:, t, :], ident)
            nc.vector.tensor_copy(dst[:, row0 : row0 + nrows], ops[:, : n * P])

        def transpose_512(src_ap, dst, r):
            transpose_rows(src_ap, dst, r * 512, 512)

        # PE p-state warm-up: the tensor engine only reaches full clock
        # after ~3us of continuous execution. Dummy identity transposes —
        # interleaved with the staging transposes so the ramp is never
        # reset by a DMA wait — bring it to speed so mega 0 runs at
        # 2.4 GHz instead of 1.2.
        warm = grp_ps.tile([P, 1536], f32, tag="grp")

        def warmup(k):
            for w in range(k):
                nc.tensor.transpose(warm[:, 0:P], ident, ident)

        # K rows 0-255 (chunks 0-1, all the leading short group needs),
        # then Q mega 0, then the rest of K; V pieces trail the early K
        # DMAs since AV lags the pipeline by two groups.
        warmup(6)
        transpose_rows(k_d, kt, 0, 256)
        warmup(1)
        transpose_rows(q_d, qt, 0, 512)
        warmup(1)
        transpose_rows(k_d, kt, 256, 256)
        for r in range(1, S // 512):
            if r >= 2:
                load_v_piece(r - 2)
            transpose_512(k_d, kt, r)
        load_v_piece(6)
        load_v_piece(7)

        pending_ot = None
        pending_fold = None
        pending_epi_a = None
        pending_epi_b = None
        av_queue = []       # (is_last_of_mega, av_fn) — flushed 2 deep
        acc_holder = {}     # mega -> acc PSUM tile, created in flush order

        def flush_av():
            nonlocal pending_ot
            is_last, fn = av_queue.pop(0)
            fn()
            if is_last:
                # This was some mega's final AV write: drain its acc
                # (O^T copy) and only then create the next mega's acc
                # tile, keeping the single-buffer acc pool in order.
                if pending_ot is not None:
                    pending_ot()
                    pending_ot = None
                nm = max(acc_holder) + 1
                if nm < N_MEGA:
                    acc_holder[nm] = acc_ps.tile(
                        [P, QMEGA], f32, tag="acc", name=f"acc{nm}"
                    )

        for m in range(N_MEGA):
            qs = slice(m * QMEGA, (m + 1) * QMEGA)
            if m == 0:
                acc_holder[0] = acc_ps.tile(
                    [P, QMEGA], f32, tag="acc", name="acc0"
                )
            acc_d = accp.tile([P, 1536], bf16, tag="acc_d")
            et0 = None

            for g in range(N_GRP):
                width = GRP_OF[g] * 512
                # --- PE: QK matmuls for group g (ahead of older AVs) ---
                gp = grp_ps.tile([P, 1536], f32, tag="grp")
                for j in range(GRP_OF[g]):
                    kc = GRP_OFF[g] + j
                    nc.tensor.matmul(
                        gp[:, j * 512 : (j + 1) * 512],
                        lhsT=kt[:, kc * P : (kc + 1) * P],
                        rhs=qt[:, qs],
                        start=True,
                        stop=True,
                    )
                # --- ACT: exp -> bf16 SBUF ---
                et = etp.tile([P, 1536], bf16, tag="et")
                nc.scalar.activation(et[:, :width], gp[:, :width], Exp)
                # --- softmax partial sums: bf16 DVE accumulate ---
                # (the short lead group's E is folded in at g==2, once the
                # accumulator has been initialized from two full groups)
                if g == 0:
                    et0 = et
                elif g == 1:
                    et1 = et
                elif g == 2:
                    nc.vector.tensor_add(acc_d, et1, et)
                    nc.vector.tensor_add(
                        acc_d[:, :1024], acc_d[:, :1024], et0[:, :1024]
                    )
                else:
                    nc.vector.tensor_add(acc_d, acc_d, et)

                def make_av(m, g, et):
                    def av():
                        acc = acc_holder[m]
                        for j in range(GRP_OF[g]):
                            kc = GRP_OFF[g] + j
                            nc.tensor.matmul(
                                acc,
                                lhsT=vtr[:, kc, :],
                                rhs=et[:, j * 512 : (j + 1) * 512],
                                start=(kc == 0),
                                stop=(kc == N_CHUNK - 1),
                                skip_group_check=True,
                            )

                    return av

                # Two-deep AV lag: QK(g) and QK(g+1) both sit ahead of
                # AV(g-2) in the PE stream, so the exp pipeline never
                # starves across mega boundaries.
                av_queue.append((g == N_GRP - 1, make_av(m, g, et)))
                if len(av_queue) > 2:
                    flush_av()

                # --- per-mega specials slotted into PE gaps ---
                if g == 2 and pending_fold is not None:
                    pending_fold()
                    pending_fold = None
                if g == 4 and pending_epi_a is not None:
                    pending_epi_a()
                    pending_epi_a = None
                if g == 6 and pending_epi_b is not None:
                    pending_epi_b()
                    pending_epi_b = None
                if g == 8 and m + 1 < N_MEGA:
                    transpose_512(q_d, qt, m + 1)

            def make_stages(m, acc_d):
                state = {}

                def ot_copy():
                    ot_sb = outp.tile([P, QMEGA], f32, tag="ot")
                    nc.vector.tensor_copy(ot_sb, acc_holder[m])
                    state["ot"] = ot_sb

                def fold():
                    # partition-reduce the bf16 partials: 3 ones-matmuls
                    sums = o_ps.tile([1, QMEGA], f32, tag="ops")
                    for j in range(3):
                        nc.tensor.matmul(
                            sums,
                            lhsT=ones_bf,
                            rhs=acc_d[:, j * 512 : (j + 1) * 512],
                            start=(j == 0),
                            stop=(j == 2),
                            skip_group_check=True,
                        )
                    sums_sb = smallp.tile([1, QMEGA], f32, tag="sums_sb")
                    nc.vector.tensor_copy(sums_sb, sums)
                    state["sums"] = sums_sb

                def epi_a():
                    sums_sb = state["sums"]
                    # 1/sums: [1, 512] -> [128, 4] per-partition scalars
                    rt = o_ps.tile([P, 4], f32, tag="ops")
                    for t in range(4):
                        nc.tensor.transpose(
                            rt[:, t : t + 1],
                            sums_sb[0:1, t * P : (t + 1) * P],
                            ident[0:1, 0:1],
                        )
                    recip = smallp.tile([P, 4], f32, tag="recip")
                    nc.vector.reciprocal(recip, rt)
                    state["recip"] = recip

                def epi_b():
                    ot_sb = state["ot"]
                    recip = state["recip"]
                    # O^T -> O, normalize, +V, one batched store per mega
                    ops2 = o_ps.tile([P, 512], f32, tag="ops")
                    for t in range(4):
                        nc.tensor.transpose(
                            ops2[:, t * P : (t + 1) * P],
                            ot_sb[:, t * P : (t + 1) * P],
                            ident,
                        )
                    o_sb = outp.tile([P, 4, P], f32, tag="osb")
                    for t in range(4):
                        nc.vector.scalar_tensor_tensor(
                            o_sb[:, t, :],
                            ops2[:, t * P : (t + 1) * P],
                            recip[:, t : t + 1],
                            vt[:, m * 4 + t, :],
                            mybir.AluOpType.mult,
                            mybir.AluOpType.add,
                        )
                    out_re = o_d[m * QMEGA : (m + 1) * QMEGA, :].rearrange(
                        "(n p) d -> p n d", p=P
                    )
                    if m == N_MEGA - 1:
                        # tail: split the final store across two issuing
                        # engines so the transfers overlap
                        nc.sync.dma_start(out=out_re[:, 0:2, :], in_=o_sb[:, 0:2, :])
                        nc.scalar.dma_start(out=out_re[:, 2:4, :], in_=o_sb[:, 2:4, :])
                    else:
                        nc.sync.dma_start(out=out_re, in_=o_sb)

                return ot_copy, fold, epi_a, epi_b

            pending_ot, pending_fold, pending_epi_a, pending_epi_b = make_stages(
                m, acc_d
            )
        while av_queue:
            flush_av()
        if pending_ot is not None:
            pending_ot()
        pending_fold()
        pending_epi_a()
        pending_epi_b()

    _split_excess_waits(nc)
    _NC_CACHE["nc"] = nc
    return nc


def kernel_run(inputs, trace=False):
    from concourse.bass_utils import run_bass_kernel_spmd

    query = np.ascontiguousarray(inputs["query"], dtype=np.float32)
    key = np.ascontiguousarray(inputs["key"], dtype=np.float32)
    value = np.ascontiguousarray(inputs["value"], dtype=np.float32)
    assert query.shape == (B, S, D), query.shape

    nc = _build_nc()
    in_maps = [
        {
            "query": np.ascontiguousarray(query[c]),
            "key": np.ascontiguousarray(key[c]),
            "value": np.ascontiguousarray(value[c]),
        }
        for c in range(N_CORES)
    ]
    res = run_bass_kernel_spmd(nc, in_maps, list(range(N_CORES)), trace=trace)
    out = np.stack([res.results[c]["out"] for c in range(N_CORES)], axis=0)
    return out.astype(np.float32), res


def kernel(**inputs) -> np.ndarray:
    out, _ = kernel_run(inputs, trace=False)
    return out


# revision 34
# speedup vs baseline: 1.2550x; 1.0270x over previous
"""Trainium2 Bass kernel for batched dense attention.

Problem: query/key/value [B=8, S=4096, D=128] fp32.
    logits = q @ k^T          (no scaling)
    attn   = softmax(logits, axis=-1)
    out    = attn @ v + v

Sharding: batch B=8 across the 8 NeuronCores (data parallel, no comms).

Per-core algorithm ("transposed attention", softmax over the partition axis),
v3 — ACT-saturating software pipeline:
    For each 512-query mega-block m, for each group g of 3 key-chunks
    (last group has 2):
      PSUM[k128, q1536] = K^T chunks . Q^T        (f32r matmuls, 3 banks)
      E^T group         = exp(PSUM) -> SBUF bf16  (one ACT instr, 1536 free)
      softmax partials: one bf16 DVE tensor_add of the E^T group into a
        running [128, 1536] accumulator (bf16 2x DVE mode);
        per-mega tail: 3 ones-matmuls partition-reduce it into PSUM.
      O^T[d, q512] += V^T . E^T      (bf16 PE matmuls, PSUM-accumulated)
    The PE stream is software-pipelined as QK(g+1) ; AV(g) so the Scalar
    (ACT) engine — the bottleneck at ~1.5us per 1536-wide exp — never
    waits behind AV's dependency on exp(g).
    Epilogue per mega (slotted into the next mega's PE gaps):
      out[q, d] = (O^T)^T * (1/sums)[q] + V[q, :]

Max-subtraction is skipped: logits ~ N(0, 128), |logit| < ~70 w.h.p., so
exp() stays inside fp32/bf16 range and the softmax ratio is unaffected.
bf16 E/V keep the end-to-end rel error ~1e-3 (gate is 2e-2).
"""

import numpy as np

B, S, D = 8, 4096, 128
N_CORES = 8
P = 128                 # partitions
QMEGA = 512             # queries per mega-block
N_MEGA = S // QMEGA     # 8
N_CHUNK = S // P        # 32 key chunks per core
# groups of key-chunks per PSUM/exp step. The short group leads each
# mega: the first QK then only needs the first 256 K rows (fast start),
# and every mega-boundary exp window is a full-width 1536 one, wide
# enough to cover the AV+QK tensor work scheduled under it.
GRP_OF = [2] + [3] * 10
N_GRP = len(GRP_OF)     # 11
GRP_OFF = [sum(GRP_OF[:i]) for i in range(N_GRP)]

_NC_CACHE = {}


def _patch_tile_drain(tile_mod):
    """Workaround for this walrus build rejecting >1-2 sem waits on the Tile
    tail Drain ("Too many sync wait commands"): spread the drain's waits
    across single-wait NOPs on the sync engine first."""
    if getattr(tile_mod.TileContext, "_drain_patched", False):
        return
    from concourse.vector_clock import ScopedClock
    from concourse import mybir

    def _drain_and_barrier(self, tick_clock, wait_clock):
        nc = self.nc
        probe = nc.sync.nop()
        wait_clock.add_sem_waits(
            probe.ins, ScopedClock({None: tick_clock.global_clock})
        )
        waits = (
            list(probe.ins.sync_info.on_wait or []) if probe.ins.sync_info else []
        )
        if probe.ins.sync_info is not None:
            probe.ins.sync_info.on_wait.clear()
        for w in waits:
            n = nc.sync.nop()
            n.ins.sync_info = mybir.SyncInfo(on_wait=[w], on_update=[])
        nc.sync.drain()

        nc.all_engine_barrier()
        assert self.sems is not None
        popped = nc._tile_sem_poison_stack.pop()
        assert popped is self._sem_poison
        nc.clear_and_free_semaphores(list(self.sems.allocated().values()))
        nc.all_engine_barrier()

    tile_mod.TileContext._drain_and_barrier = _drain_and_barrier
    tile_mod.TileContext._drain_patched = True


# This walrus build fits only ONE sync wait per emitted instruction
# (S3_LW matmuls and PSEUDO_DMA reject 2; Drain rejects 3) — cap at 1
# everywhere and carry excess waits on preceding same-engine NoOps.
_MAX_WAITS = 1
_MAX_WAITS_MATMUL = 1


def _split_excess_waits(nc):
    """Post-scheduling legalization: any instruction carrying more than
    the walrus per-instruction sync-wait limit gets same-engine NoOps
    inserted before it that carry the excess waits (the NX executes them
    in program order)."""
    from concourse import mybir

    uid = 0
    for fn in nc.m.functions:
        for bb in fn.blocks:
            new_insts = []
            for inst in bb.instructions:
                limit = (
                    _MAX_WAITS_MATMUL
                    if isinstance(inst, mybir.InstMatmult)
                    else _MAX_WAITS
                )
                si = inst.sync_info
                waits = list(si.on_wait) if (si and si.on_wait) else []
                if len(waits) > limit:
                    extra, keep = waits[:-limit], waits[-limit:]
                    for i in range(0, len(extra), _MAX_WAITS):
                        chunk = extra[i : i + _MAX_WAITS]
                        nop = mybir.InstNoOp(
                            name=f"I-waitsplit-{uid}", ins=[], outs=[]
                        )
                        uid += 1
                        nop.engine = inst.engine
                        nop.sync_info = mybir.SyncInfo(
                            on_wait=list(chunk), on_update=[]
                        )
                        new_insts.append(nop)
                    si.on_wait.clear()
                    si.on_wait.extend(keep)
                new_insts.append(inst)
            bb.instructions = new_insts


def _build_nc():
    if "nc" in _NC_CACHE:
        return _NC_CACHE["nc"]
    from contextlib import ExitStack

    import concourse.bass as bass
    import concourse.tile as tile
    from concourse import mybir
    from concourse.masks import make_identity

    _patch_tile_drain(tile)

    f32 = mybir.dt.float32
    f32r = mybir.dt.float32r
    bf16 = mybir.dt.bfloat16
    Exp = mybir.ActivationFunctionType.Exp

    nc = bass.Bass()
    q_d = nc.declare_dram_parameter("query", [S, D], f32, isOutput=False)
    k_d = nc.declare_dram_parameter("key", [S, D], f32, isOutput=False)
    v_d = nc.declare_dram_parameter("value", [S, D], f32, isOutput=False)
    o_d = nc.declare_dram_parameter("out", [S, D], f32, isOutput=True)

    with tile.TileContext(nc) as tc, ExitStack() as ctx:
        const = ctx.enter_context(tc.tile_pool(name="const", bufs=1))
        big = ctx.enter_context(tc.tile_pool(name="big", bufs=1))
        stage = ctx.enter_context(tc.tile_pool(name="stage", bufs=3))
        etp = ctx.enter_context(tc.tile_pool(name="et", bufs=6))
        accp = ctx.enter_context(tc.tile_pool(name="accp", bufs=2))
        outp = ctx.enter_context(tc.tile_pool(name="outp", bufs=6))
        smallp = ctx.enter_context(tc.tile_pool(name="small", bufs=4))
        # PSUM: gp 2x6KB + acc 1x2KB + shared transpose/sums 1x2KB = 16KB
        grp_ps = ctx.enter_context(tc.tile_pool(name="grp_ps", bufs=2, space="PSUM"))
        acc_ps = ctx.enter_context(tc.tile_pool(name="acc_ps", bufs=1, space="PSUM"))
        o_ps = ctx.enter_context(tc.tile_pool(name="o_ps", bufs=1, space="PSUM"))

        ident = const.tile([P, P], f32)
        make_identity(nc, ident)
        ones_f32 = const.tile([P, 1], f32)
        nc.vector.memset(ones_f32, 1.0)
        ones_bf = const.tile([P, 1], bf16)
        nc.vector.tensor_copy(ones_bf, ones_f32)

        # V resident in natural layout: vt[p, n, d] = V[n*128 + p, d]
        # (used for the +V epilogue), and vtr bf16 for the AV matmuls
        # (cast on the otherwise-idle Pool engine).
        vt = big.tile([P, N_CHUNK, P], f32)
        vtr = big.tile([P, N_CHUNK, P], bf16)
        v_re = v_d.rearrange("(n p) d -> p n d", p=P)

        def load_v_piece(i):
            sl = slice(i * 4, (i + 1) * 4)
            nc.sync.dma_start(out=vt[:, sl, :], in_=v_re[:, sl, :])
            nc.gpsimd.tensor_copy(vtr[:, sl, :], vt[:, sl, :])

        # K^T / Q^T [d, s] via PE transposes of natural [s, d] tiles.
        qt = big.tile([P, S], f32r)
        kt = big.tile([P, S], f32r)

        def stage_rows(src_ap, row0, nrows, dma_engine=None):
            n = nrows // P
            st = stage.tile([P, n, P], f32, tag="stage", name=f"st{row0}")
            (dma_engine or nc.sync).dma_start(
                out=st,
                in_=src_ap[row0 : row0 + nrows, :].rearrange(
                    "(n p) d -> p n d", p=P
                ),
            )
            return st

        def transpose_staged(st, dst, row0, nrows):
            n = nrows // P
            ops = o_ps.tile([P, 512], f32, tag="ops", name=f"ops{row0}")
            for t in range(n):
                nc.tensor.transpose(ops[:, t * P : (t + 1) * P], st[:, t, :], ident)
            nc.vector.tensor_copy(dst[:, row0 : row0 + nrows], ops[:, : n * P])

        def transpose_rows(src_ap, dst, row0, nrows, dma_engine=None):
            """dst[:, row0:row0+nrows] = src_ap[row0:row0+nrows, :].T"""
            transpose_staged(
                stage_rows(src_ap, row0, nrows, dma_engine), dst, row0, nrows
            )

        def transpose_512(src_ap, dst, r):
            transpose_rows(src_ap, dst, r * 512, 512)

        # The first loads go out in parallel on two DGE rings: K rows
        # 0-255 + slab 1 on sync, Q mega 0 + K rows 256-511 on scalar
        # (which is otherwise idle until its activation-table load).
        st_ka = stage_rows(k_d, 0, 256)
        st_q0 = stage_rows(q_d, 0, 512, dma_engine=nc.scalar)
        st_kb = stage_rows(k_d, 256, 256, dma_engine=nc.scalar)
        st_k1 = stage_rows(k_d, 512, 512)

        # PE p-state warm-up: the tensor engine only reaches full clock
        # after ~3us of continuous execution. Dummy identity transposes —
        # interleaved with the staging transposes so the ramp is never
        # reset by a DMA wait — bring it to speed so mega 0 runs at
        # 2.4 GHz instead of 1.2.
        warm = grp_ps.tile([P, 1536], f32, tag="grp")

        def warmup(k):
            for w in range(k):
                nc.tensor.transpose(warm[:, 0:P], ident, ident)

        warmup(5)
        transpose_staged(st_ka, kt, 0, 256)
        warmup(1)
        transpose_staged(st_q0, qt, 0, 512)
        transpose_staged(st_kb, kt, 256, 256)
        transpose_staged(st_k1, kt, 512, 512)
        for r in range(2, S // 512):
            load_v_piece(r - 2)
            transpose_512(k_d, kt, r)
        load_v_piece(6)
        load_v_piece(7)

        pending_ot = None
        pending_stages = []
        av_queue = []       # (is_last_of_mega, av_fn) — flushed 2 deep
        acc_holder = {}     # mega -> acc PSUM tile, created in flush order

        def flush_av():
            nonlocal pending_ot
            is_last, fn = av_queue.pop(0)
            fn()
            if is_last:
                # This was some mega's final AV write: drain its acc
                # (O^T copy) and only then create the next mega's acc
                # tile, keeping the single-buffer acc pool in order.
                if pending_ot is not None:
                    pending_ot()
                    pending_ot = None
                nm = max(acc_holder) + 1
                if nm < N_MEGA:
                    acc_holder[nm] = acc_ps.tile(
                        [P, QMEGA], f32, tag="acc", name=f"acc{nm}"
                    )

        for m in range(N_MEGA):
            qs = slice(m * QMEGA, (m + 1) * QMEGA)
            if m == 0:
                acc_holder[0] = acc_ps.tile(
                    [P, QMEGA], f32, tag="acc", name="acc0"
                )
            acc_d = accp.tile([P, 1536], bf16, tag="acc_d")
            et0 = None

            # One small "special" per group slot: the previous mega's
            # fold/epilogue plus the next mega's Q^T staging, spread so
            # no single exp window absorbs a burst of extra PE work.
            slots = list(pending_stages)
            pending_stages = []
            if m + 1 < N_MEGA:
                qst = {}

                def make_qstage(nm, qst):
                    def qdma():
                        qst["st"] = stage_rows(q_d, nm * 512, 512)

                    def qt01():
                        qst["ops"] = o_ps.tile(
                            [P, 512], f32, tag="ops", name=f"qops{nm}"
                        )
                        for t in range(2):
                            nc.tensor.transpose(
                                qst["ops"][:, t * P : (t + 1) * P],
                                qst["st"][:, t, :],
                                ident,
                            )

                    def qt23():
                        for t in range(2, 4):
                            nc.tensor.transpose(
                                qst["ops"][:, t * P : (t + 1) * P],
                                qst["st"][:, t, :],
                                ident,
                            )
                        nc.vector.tensor_copy(
                            qt[:, nm * 512 : (nm + 1) * 512], qst["ops"]
                        )

                    return qdma, qt01, qt23

                qdma, qt01, qt23 = make_qstage(m + 1, qst)
                slots.insert(min(2, len(slots)), qdma)
                slots.append(qt01)
                slots.append(qt23)

            for g in range(N_GRP):
                width = GRP_OF[g] * 512
                # --- PE: QK matmuls for group g (ahead of older AVs) ---
                gp = grp_ps.tile([P, 1536], f32, tag="grp")
                for j in range(GRP_OF[g]):
                    kc = GRP_OFF[g] + j
                    nc.tensor.matmul(
                        gp[:, j * 512 : (j + 1) * 512],
                        lhsT=kt[:, kc * P : (kc + 1) * P],
                        rhs=qt[:, qs],
                        start=True,
                        stop=True,
                    )
                # --- ACT: exp -> bf16 SBUF ---
                et = etp.tile([P, 1536], bf16, tag="et")
                nc.scalar.activation(et[:, :width], gp[:, :width], Exp)
                # --- softmax partial sums: bf16 DVE accumulate ---
                # (the short lead group's E is folded in at g==2, once the
                # accumulator has been initialized from two full groups)
                if g == 0:
                    et0 = et
                elif g == 1:
                    et1 = et
                elif g == 2:
                    nc.vector.tensor_add(acc_d, et1, et)
                    nc.vector.tensor_add(
                        acc_d[:, :1024], acc_d[:, :1024], et0[:, :1024]
                    )
                else:
                    nc.vector.tensor_add(acc_d, acc_d, et)

                def make_av(m, g, et):
                    def av():
                        acc = acc_holder[m]
                        for j in range(GRP_OF[g]):
                            kc = GRP_OFF[g] + j
                            nc.tensor.matmul(
                                acc,
                                lhsT=vtr[:, kc, :],
                                rhs=et[:, j * 512 : (j + 1) * 512],
                                start=(kc == 0),
                                stop=(kc == N_CHUNK - 1),
                                skip_group_check=True,
                            )

                    return av

                # Two-deep AV lag: QK(g) and QK(g+1) both sit ahead of
                # AV(g-2) in the PE stream, so the exp pipeline never
                # starves across mega boundaries.
                av_queue.append((g == N_GRP - 1, make_av(m, g, et)))
                if len(av_queue) > 2:
                    flush_av()

                # --- per-mega specials: one slot per group ---
                # (mega 0 defers its slots behind the prologue K staging,
                # which shares the transpose PSUM bank)
                if g >= (6 if m == 0 else 1) and slots:
                    slots.pop(0)()

            def make_stages(m, acc_d):
                state = {}

                def ot_copy():
                    ot_sb = outp.tile([P, QMEGA], f32, tag="ot")
                    nc.vector.tensor_copy(ot_sb, acc_holder[m])
                    state["ot"] = ot_sb

                def fold12():
                    # partition-reduce the bf16 partials: 3 ones-matmuls
                    state["psums"] = o_ps.tile(
                        [1, QMEGA], f32, tag="ops", name=f"sums{m}"
                    )
                    for j in range(2):
                        nc.tensor.matmul(
                            state["psums"],
                            lhsT=ones_bf,
                            rhs=acc_d[:, j * 512 : (j + 1) * 512],
                            start=(j == 0),
                            stop=False,
                            skip_group_check=True,
                        )

                def fold3():
                    nc.tensor.matmul(
                        state["psums"],
                        lhsT=ones_bf,
                        rhs=acc_d[:, 1024:1536],
                        start=False,
                        stop=True,
                        skip_group_check=True,
                    )
                    sums_sb = smallp.tile([1, QMEGA], f32, tag="sums_sb")
                    nc.vector.tensor_copy(sums_sb, state["psums"])
                    state["sums"] = sums_sb

                def rt_recip():
                    sums_sb = state["sums"]
                    # 1/sums: [1, 512] -> [128, 4] per-partition scalars
                    rt = o_ps.tile([P, 4], f32, tag="ops", name=f"rt{m}")
                    for t in range(4):
                        nc.tensor.transpose(
                            rt[:, t : t + 1],
                            sums_sb[0:1, t * P : (t + 1) * P],
                            ident[0:1, 0:1],
                        )
                    recip = smallp.tile([P, 4], f32, tag="recip")
                    nc.vector.reciprocal(recip, rt)
                    state["recip"] = recip

                def make_ops2(t0):
                    def ops2_half():
                        if "ops2" not in state:
                            state["ops2"] = o_ps.tile(
                                [P, 512], f32, tag="ops", name=f"ops2_{m}"
                            )
                        for t in (t0, t0 + 1):
                            nc.tensor.transpose(
                                state["ops2"][:, t * P : (t + 1) * P],
                                state["ot"][:, t * P : (t + 1) * P],
                                ident,
                            )

                    return ops2_half

                def stt_store():
                    recip = state["recip"]
                    o_sb = outp.tile([P, 4, P], f32, tag="osb")
                    for t in range(4):
                        nc.vector.scalar_tensor_tensor(
                            o_sb[:, t, :],
                            state["ops2"][:, t * P : (t + 1) * P],
                            recip[:, t : t + 1],
                            vt[:, m * 4 + t, :],
                            mybir.AluOpType.mult,
                            mybir.AluOpType.add,
                        )
                    out_re = o_d[m * QMEGA : (m + 1) * QMEGA, :].rearrange(
                        "(n p) d -> p n d", p=P
                    )
                    if m == N_MEGA - 1:
                        # tail: split the final store across two issuing
                        # engines so the transfers overlap
                        nc.sync.dma_start(out=out_re[:, 0:2, :], in_=o_sb[:, 0:2, :])
                        nc.scalar.dma_start(out=out_re[:, 2:4, :], in_=o_sb[:, 2:4, :])
                    else:
                        nc.sync.dma_start(out=out_re, in_=o_sb)

                return ot_copy, [
                    fold12,
                    fold3,
                    rt_recip,
                    make_ops2(0),
                    make_ops2(2),
                    stt_store,
                ]

            pending_ot, pending_stages = make_stages(m, acc_d)
        while av_queue:
            flush_av()
        if pending_ot is not None:
            pending_ot()
        for s in pending_stages:
            s()

    _split_excess_waits(nc)
    _NC_CACHE["nc"] = nc
    return nc


def kernel_run(inputs, trace=False):
    from concourse.bass_utils import run_bass_kernel_spmd

    query = np.ascontiguousarray(inputs["query"], dtype=np.float32)
    key = np.ascontiguousarray(inputs["key"], dtype=np.float32)
    value = np.ascontiguousarray(inputs["value"], dtype=np.float32)
    assert query.shape == (B, S, D), query.shape

    nc = _build_nc()
    in_maps = [
        {
            "query": np.ascontiguousarray(query[c]),
            "key": np.ascontiguousarray(key[c]),
            "value": np.ascontiguousarray(value[c]),
        }
        for c in range(N_CORES)
    ]
    res = run_bass_kernel_spmd(nc, in_maps, list(range(N_CORES)), trace=trace)
    out = np.stack([res.results[c]["out"] for c in range(N_CORES)], axis=0)
    return out.astype(np.float32), res


def kernel(**inputs) -> np.ndarray:
    out, _ = kernel_run(inputs, trace=False)
    return out
